# revision 1
# baseline (speedup 1.0000x reference)
"""Trainium2 Bass kernel for nn_CombinedCS (FISTA compressed-sensing recon).

Self-contained: hardcodes shapes (B=16, H=W=320), shards batch over 8 cores
(2 images per core), runs the full 15-iteration FISTA loop SBUF-resident.

Math plan (validated vs reference):
  - centered 2D FFT as two PE matmul stages against the DFT matrix F
    (transpose-free: data is always lhsT, F^T always rhs)
  - data fidelity uses the binary mask identity m*(m*F z - y) = m*F z - m*y,
    so z_step = z - iF(m*F z) + c0 with c0 = iF(m*y) precomputed on host
  - TV prox (5 Chambolle iters): h-direction div/grad as PE left-mults by
    BANDED bidiagonal matrices (only the 2 contraction tiles that carry the
    band); w-direction via shifted free-dim views with zero guard columns;
    inner loop in bf16 (2x DVE rate), duals carry a lam scaling
  - 3-level Haar DWT: w-step unnormalized (a+b, a-b) on DVE/Pool, h-step as
    PE left-mult by orthonormal Haar matrix; detail soft-threshold via
    x - clamp(x, -t, t); inverse folds the w-step 1/2 into the h-step matrix

Layout P6: one complex image (2 ch x 320 x 320) packs into
[128 partitions, 6 blocks, 320]; channel ch occupies blocks 3ch..3ch+2
with h = 128*qb + p (block 3ch+2 uses p<64; its p>=64 "dead" region is kept
zero/finite so ops can process whole channels as single [128,3,w] views).

Scheduling: the two images per core are software-pipelined — image (i+1)'s
PE-heavy FFT phase is interleaved chunk-by-chunk with image i's DVE-heavy
TV/DWT/momentum phases. PSUM: pool psa = [128,3,512] x 2 bufs (6 banks, all
per-channel matmul groups), psb = [128,4,256] (2 banks, DWT level 2/3).
Elementwise work is spread across DVE ("v"), Pool ("g", TensorTensor/copy
only, no PSUM), and Act ("a", activations/copies).
"""
import math
import os

import ml_dtypes
import numpy as np

H = W = 320
B = 16
NCORES = 8
IMGS = B // NCORES  # 2
LAM_TV = 0.005
LAM_WAV = 0.005
TAU = 0.25
TV_ITERS = 5
LEVELS = 3
MAX_ITER = int(os.environ.get("CS_ITERS", "15"))
SKIP_TV = os.environ.get("CS_SKIP_TV", "0") == "1"
SKIP_DWT = os.environ.get("CS_SKIP_DWT", "0") == "1"
S2 = math.sqrt(2.0)

# layouts: per ch, list of (p0, p1, q, r0, r1): matrix rows r0..r1 live at
# partitions p0..p1 of block q. All tiles base-0 (matmul dst requirement).
P6D = {
    0: [(0, 128, 0, 0, 128), (0, 128, 1, 128, 256), (0, 64, 2, 256, 320)],
    1: [(0, 128, 3, 0, 128), (0, 128, 4, 128, 256), (0, 64, 5, 256, 320)],
}
P6C = [(0, 128, 0, 0, 128), (0, 128, 1, 128, 256), (0, 64, 2, 256, 320)]
L2D = {
    0: [(0, 128, 0, 0, 128), (0, 32, 1, 128, 160)],
    1: [(0, 128, 2, 0, 128), (0, 32, 3, 128, 160)],
}
L2C = [(0, 128, 0, 0, 128), (0, 32, 1, 128, 160)]
L3D = {
    0: [(0, 64, 0, 0, 64), (0, 16, 1, 64, 80)],
    1: [(0, 64, 2, 0, 64), (0, 16, 3, 64, 80)],
}
L3C = [(0, 64, 0, 0, 64), (0, 16, 1, 64, 80)]


def _dft_mats():
    I = np.eye(H, dtype=np.complex128)
    F = np.fft.fftshift(
        np.fft.fft(np.fft.ifftshift(I, axes=0), axis=0, norm="ortho"), axes=0
    )
    G = np.conj(F).T
    return F, G


def _tv_mats():
    Dd = np.zeros((H, H))
    Dd[0, 0] = 1.0
    for h in range(1, H - 1):
        Dd[h, h] = 1.0
        Dd[h, h - 1] = -1.0
    Dd[H - 1, H - 2] = -1.0
    Dg = np.zeros((H, H))
    for h in range(H - 1):
        Dg[h, h] = -1.0
        Dg[h, h + 1] = 1.0
    return Dd, Dg


def _haar_mat(n):
    Wm = np.zeros((n, n))
    hn = n // 2
    c = 1.0 / S2
    for i in range(hn):
        Wm[i, 2 * i] = c
        Wm[i, 2 * i + 1] = c
        Wm[hn + i, 2 * i] = c
        Wm[hn + i, 2 * i + 1] = -c
    return Wm


def _momentum_coeffs():
    t = 1.0
    out = []
    for _ in range(MAX_ITER):
        t_new = (1.0 + math.sqrt(1.0 + 4.0 * t * t)) / 2.0
        out.append((t - 1.0) / t_new)
        t = t_new
    return out


def _pack_p6(x):
    """x: (2, 320, 320) -> (128, 6, 320), zero-padded dead region."""
    out = np.zeros((128, 6, 320), dtype=x.dtype)
    for ch in range(2):
        out[:, 3 * ch + 0] = x[ch, 0:128]
        out[:, 3 * ch + 1] = x[ch, 128:256]
        out[0:64, 3 * ch + 2] = x[ch, 256:320]
    return out


def _unpack_p6(p):
    out = np.zeros((2, 320, 320), dtype=p.dtype)
    for ch in range(2):
        out[ch, 0:128] = p[:, 3 * ch + 0]
        out[ch, 128:256] = p[:, 3 * ch + 1]
        out[ch, 256:320] = p[0:64, 3 * ch + 2]
    return out


def _host_consts():
    F, G = _dft_mats()
    Dd, Dg = _tv_mats()
    W1, W2, W3 = _haar_mat(320), _haar_mat(160), _haar_mat(80)
    f32 = np.float32
    bf16 = ml_dtypes.bfloat16
    return {
        "ftr": F.real.T.astype(f32), "fti": F.imag.T.astype(f32),
        "ftin": (-F.imag.T).astype(f32),
        "ifr": G.real.T.astype(f32), "ifi": G.imag.T.astype(f32),
        "ifin": (-G.imag.T).astype(f32),
        "ddt": Dd.T.astype(bf16), "dgt": Dg.T.astype(bf16),
        "dgtf": Dg.T.astype(f32),
        "w1t": W1.T.astype(f32), "w1h": (0.5 * W1).astype(f32),
        "w2t": W2.T.astype(f32), "w2h": (0.5 * W2).astype(f32),
        "w3t": W3.T.astype(f32), "w3h": (0.5 * W3).astype(f32),
    }


def _ifft2c_np(x):
    # x: (2, H, W) real/imag -> centered inverse 2D FFT, same layout
    xc = x[0] + 1j * x[1]
    ic = np.fft.fftshift(
        np.fft.ifft2(np.fft.ifftshift(xc, axes=(-2, -1)), norm="ortho"),
        axes=(-2, -1))
    return np.stack([ic.real, ic.imag], axis=0).astype(np.float32)


def build_in_maps(y, mask):
    """Per-core input maps. c0_i = iF(mask*y) exploits the binary mask:
    mask*(mask*F(z) - y) = mask*F(z) - mask*y, so the data-fidelity step is
    z - iF(mask*F(z)) + c0 with c0 constant across iterations."""
    c = _host_consts()
    in_maps = []
    for core in range(NCORES):
        m = dict(c)
        m["zz"] = np.zeros((128, 6, 322), dtype=ml_dtypes.bfloat16)
        m["zzf"] = np.zeros((128, 6, 320), dtype=np.float32)
        for i in range(IMGS):
            b = core * IMGS + i
            mpair = np.broadcast_to(mask[b], (2, 320, 320)).astype(np.float32)
            m[f"y{i}"] = _pack_p6(y[b])
            m[f"c0{i}"] = _pack_p6(_ifft2c_np(mask[b] * y[b]))
            m[f"mk{i}"] = _pack_p6(mpair).astype(ml_dtypes.bfloat16)
        in_maps.append(m)
    return in_maps


def _copy_segs(src_lay, dst_lay, nrows):
    out = {}
    for ch in (0, 1):
        def locate(lay, r):
            for (p0, p1, q, r0, r1) in lay[ch]:
                if r0 <= r < r1:
                    return p0 + (r - r0), q, r1 - r
            raise AssertionError(r)
        segs = []
        r = 0
        while r < nrows:
            sp, sq, sleft = locate(src_lay, r)
            dp, dq, dleft = locate(dst_lay, r)
            cnt = min(sleft, dleft, nrows - r)
            segs.append((sp, sq, dp, dq, cnt))
            r += cnt
        out[ch] = segs
    return out


SEG12 = _copy_segs(P6D, L2D, 160)
SEG23 = _copy_segs(L2D, L3D, 80)


def _build_nc():
    import concourse.bacc as bacc
    import concourse.tile as tile
    import concourse.mybir as mybir
    from contextlib import ExitStack

    dt = mybir.dt
    F32, F32R, BF16 = dt.float32, dt.float32r, dt.bfloat16
    ALU = mybir.AluOpType
    AF = mybir.ActivationFunctionType

    s_tv = TAU * LAM_TV
    lam = LAM_TV
    eps_q = lam * lam * 1e-8
    lam1 = lam / s_tv
    eps1 = eps_q / (s_tv * s_tv)
    coeffs = _momentum_coeffs()
    lam_lvl = [LAM_WAV * (S2 ** (l + 1)) for l in range(LEVELS)]

    nc = bacc.Bacc("TRN2", target_bir_lowering=False, debug=False,
                   num_devices=NCORES)

    dr = {}
    for name in ("ftr", "fti", "ftin", "ifr", "ifi", "ifin", "w1t", "w1h"):
        dr[name] = nc.dram_tensor(name, [320, 320], F32R, kind="ExternalInput").ap()
    for name in ("w2t", "w2h"):
        dr[name] = nc.dram_tensor(name, [160, 160], F32R, kind="ExternalInput").ap()
    for name in ("w3t", "w3h"):
        dr[name] = nc.dram_tensor(name, [80, 80], F32R, kind="ExternalInput").ap()
    for name in ("ddt", "dgt"):
        dr[name] = nc.dram_tensor(name, [320, 320], BF16, kind="ExternalInput").ap()
    dr["dgtf"] = nc.dram_tensor("dgtf", [320, 320], F32R, kind="ExternalInput").ap()
    dr["zz"] = nc.dram_tensor("zz", [128, 6, 322], BF16, kind="ExternalInput").ap()
    dr["zzf"] = nc.dram_tensor("zzf", [128, 6, 320], F32R, kind="ExternalInput").ap()
    for i in range(IMGS):
        dr[f"y{i}"] = nc.dram_tensor(f"y{i}", [128, 6, 320], F32R, kind="ExternalInput").ap()
        dr[f"c0{i}"] = nc.dram_tensor(f"c0{i}", [128, 6, 320], F32, kind="ExternalInput").ap()
        dr[f"mk{i}"] = nc.dram_tensor(f"mk{i}", [128, 6, 320], BF16, kind="ExternalInput").ap()
        dr[f"xo{i}"] = nc.dram_tensor(f"xo{i}", [128, 6, 320], F32, kind="ExternalOutput").ap()

    with ExitStack() as ctx:
        tc = ctx.enter_context(tile.TileContext(nc))
        st = ctx.enter_context(tc.tile_pool(name="state", bufs=1))
        psa = ctx.enter_context(tc.tile_pool(name="psa", bufs=2, space="PSUM"))
        psb = ctx.enter_context(tc.tile_pool(name="psb", bufs=2, space="PSUM"))

        def T(tag, shape, dtp):
            return st.tile(shape, dtp, tag=tag, name=tag)

        cv = {}
        for name in ("ftr", "fti", "ftin", "ifr", "ifi", "ifin", "w1t", "w1h"):
            cv[name] = T("c_" + name, [128, 3, 320], F32R)
        for name in ("ddt", "dgt"):
            cv[name] = T("c_" + name, [128, 3, 320], BF16)
        cv["dgtf"] = T("c_dgtf", [128, 3, 320], F32R)
        for name in ("w2t", "w2h"):
            cv[name] = T("c_" + name, [128, 2, 160], F32R)
        for name in ("w3t", "w3h"):
            cv[name] = T("c_" + name, [128, 2, 80], F32R)

        def load_const(name, lay):
            for (p0, p1, q, r0, r1) in lay:
                nc.sync.dma_start(cv[name][p0:p1, q, :], dr[name][r0:r1, :])

        for name in ("ftr", "fti", "ftin", "ifr", "ifi", "ifin", "w1t", "w1h",
                     "ddt", "dgt", "dgtf"):
            load_const(name, P6C)
        for name in ("w2t", "w2h"):
            load_const(name, L2C)
        for name in ("w3t", "w3h"):
            load_const(name, L3C)

        per_img = []
        for i in range(IMGS):
            per_img.append({
                "z": T(f"z{i}", [128, 6, 320], F32R),
                "xA": T(f"xA{i}", [128, 6, 320], F32),
                "xB": T(f"xB{i}", [128, 6, 320], F32),
                "c0": T(f"c0k{i}", [128, 6, 320], F32),
                "mk": T(f"msk{i}", [128, 6, 320], BF16),
                "xcb": T(f"xcb{i}", [128, 6, 320], F32R),
            })
        sbA = T("sbA", [128, 6, 320], F32R)
        Km = T("Km", [128, 6, 320], F32R)
        xtv = T("xtv", [128, 6, 320], F32R)
        wtmp = T("wtmp", [128, 6, 320], F32R)
        Y1 = T("Y1", [128, 6, 320], F32R)
        Y2 = T("Y2", [128, 4, 160], F32R)
        Y3 = T("Y3", [128, 4, 80], F32R)
        L2t = T("L2t", [128, 4, 160], F32R)
        L3t = T("L3t", [128, 4, 80], F32R)
        wdwt = T("wdwt", [128, 6, 320], F32R)
        qx = T("qx", [128, 6, 322], BF16)
        qy = T("qy", [128, 6, 320], BF16)
        vt = T("vt", [128, 6, 320], BF16)
        tv1 = T("tv1", [128, 6, 320], BF16)
        n2 = T("n2", [128, 6, 320], BF16)

        def psA3():
            return psa.tile([128, 3, 512], F32, tag="A", name="psA")

        def psB():
            # per-channel lvl2/3 instance: 2 blocks x 256 f32 = 1 bank
            return psb.tile([128, 2, 256], F32, tag="B", name="psB")

        def mm_lvl(ps, cname, data, dlay, clay, ncols, ch):
            # per-channel level-2/3 left-mult; out blocks remapped to 0,1
            cvt = cv[cname]
            dts = dlay[ch]
            n = len(dts)
            for mi, (mp0, mp1, mq, mr0, mr1) in enumerate(dts):
                for t in range(n):
                    dp0, dp1, dq, _, _ = dts[t]
                    cp0, cp1, cq, _, _ = clay[t]
                    nc.tensor.matmul(
                        ps[mp0:mp1, mi, 0:ncols],
                        cvt[cp0:cp1, cq, mr0:mr1],
                        data[dp0:dp1, dq, 0:ncols],
                        start=(t == 0), stop=(t == n - 1))

        # zero-init via DMA (memset is not ISA-legal for f32r/bf16 here);
        # covers guard columns of qx and dead regions read via whole-channel
        # views (wtmp, vt, Y1, z)
        nc.sync.dma_start(qx[:], dr["zz"][:])
        nc.sync.dma_start(qy[:], dr["zz"][0:128, 0:6, 0:320])
        nc.sync.dma_start(vt[:], dr["zz"][0:128, 0:6, 0:320])
        nc.sync.dma_start(wtmp[:], dr["zzf"][:])
        nc.sync.dma_start(Y1[:], dr["zzf"][:])
        for i in range(IMGS):
            nc.sync.dma_start(per_img[i]["z"][:], dr["zzf"][:])
            nc.vector.memset(per_img[i]["xA"][:], 0.0)

        # ----- whole-channel views: [128, 3 blocks, w] incl. dead region -----
        def chv(t, ch, c0=0, c1=None, step=1):
            c1 = t.shape[-1] if c1 is None else c1
            if step != 1:
                return t[0:128, 3 * ch:3 * ch + 3, c0:c1:step]
            return t[0:128, 3 * ch:3 * ch + 3, c0:c1]

        def pc(t, c0=0, c1=None, step=1):
            return [chv(t, 0, c0, c1, step), chv(t, 1, c0, c1, step)]

        def psv(p, c0=0, c1=320):
            # valid sub-views only: block-2 partitions 64:128 are never
            # written by matmuls (stale bytes from the prior pool instance)
            return [p[0:128, 0:2, c0:c1], p[0:64, 2, c0:c1]]

        def chv2(t, ch, c0=0, c1=None, step=1):
            c1 = t.shape[-1] if c1 is None else c1
            b = 3 * ch
            if step != 1:
                return [t[0:128, b:b + 2, c0:c1:step],
                        t[0:64, b + 2, c0:c1:step]]
            return [t[0:128, b:b + 2, c0:c1], t[0:64, b + 2, c0:c1]]

        NO_POOL = os.environ.get("CS_NO_POOL", "0") == "1"
        ENG = {"v": nc.vector,
               "g": nc.vector if NO_POOL else nc.gpsimd,
               "a": nc.scalar}

        # ew: dispatch per-channel ops across DVE ("v") / Pool ("g") / Act
        # ("a") per 2-char pattern. Pool only supports plain TensorTensor /
        # TensorCopy (no PSUM), so other ops fall back to DVE on "g".
        def ew(fn, *views, pat="vv"):
            for i in range(len(views[0])):
                e = ENG[pat[i % len(pat)]]
                if e is nc.gpsimd and not getattr(fn, "pool_ok", False):
                    e = nc.vector
                fn(e, *[v[i] for v in views])

        def TT(op):
            fn = lambda e, o, a, b: e.tensor_tensor(o, a, b, op)
            fn.pool_ok = True
            return fn

        def STT(s, op0, op1):
            return lambda e, o, a, b: e.scalar_tensor_tensor(o, a, s, b, op0, op1)

        def SMAX(s):
            return lambda e, o, a: e.tensor_scalar_max(o, a, s)

        def SQ(e, o, a):
            if e is nc.scalar:
                e.square(o, a)
            else:
                e.tensor_tensor(o, a, a, ALU.mult)

        def CP(e, o, a):
            if e is nc.scalar:
                e.copy(o, a)
            else:
                e.tensor_copy(o, a)
        CP.pool_ok = True

        def RSQ(scale):
            return lambda e, o, a: nc.scalar.activation(
                o, a, AF.Abs_reciprocal_sqrt, scale=scale)

        def soft_views(views, lam_l, tmp_views, pat="vg"):
            # soft(x, t) = x - clamp(x, -t, t); clamp is DVE-only (Pool has
            # no TensorScalar), the subtract may go to Pool
            for i in range(len(views)):
                e = ENG[pat[i % len(pat)]]
                nc.vector.tensor_scalar(tmp_views[i], views[i], -lam_l, lam_l,
                                        ALU.max, ALU.min)
                te = e if e is not nc.scalar else nc.vector
                te.tensor_tensor(views[i], views[i], tmp_views[i], ALU.subtract)

        # ---------- per-channel matmul emitters (out blocks 0..2) ----------
        def mm_fft_ch(ps, data, terms, oc, tiles=None):
            for (mp0, mp1, mq, mr0, mr1) in (tiles or P6C):
                mml = []
                for (dch, cname) in terms[oc]:
                    cvt = cv[cname]
                    for t in range(3):
                        dp0, dp1, dq, _, _ = P6D[dch][t]
                        cp0, cp1, cq, _, _ = P6C[t]
                        mml.append((data[dp0:dp1, dq, mr0:mr1],
                                    cvt[cp0:cp1, cq, 0:320]))
                for idx, (l, r) in enumerate(mml):
                    nc.tensor.matmul(ps[mp0:mp1, mq, 0:320], l, r,
                                     start=(idx == 0), stop=(idx == len(mml) - 1))

        def mm_tv_ch(ps, cname, data, upper, ch):
            # banded bidiagonal: out tile ti needs contraction tiles
            # {ti-1, ti} (lower/Dd) or {ti, ti+1} (upper/Dg) only
            cvt = cv[cname]
            dts = P6D[ch]
            for ti in range(3):
                mp0, mp1, mq, mr0, mr1 = P6C[ti]
                need = [ti - 1, ti] if not upper else [ti, ti + 1]
                mml = []
                for t in need:
                    if t < 0 or t > 2:
                        continue
                    cp0, cp1, cq, _, _ = P6C[t]
                    dp0, dp1, dq, _, _ = dts[t]
                    mml.append((cvt[cp0:cp1, cq, mr0:mr1],
                                data[dp0:dp1, dq, 0:320]))
                for idx, (l, r) in enumerate(mml):
                    nc.tensor.matmul(ps[mp0:mp1, mq, 0:320], l, r,
                                     start=(idx == 0), stop=(idx == len(mml) - 1))

        def mm_w1_ch(ps, cname, data, ch):
            cvt = cv[cname]
            dts = P6D[ch]
            for ti in range(3):
                mp0, mp1, mq, mr0, mr1 = P6C[ti]
                for t in range(3):
                    dp0, dp1, dq, _, _ = dts[t]
                    cp0, cp1, cq, _, _ = P6C[t]
                    nc.tensor.matmul(ps[mp0:mp1, mq, 0:320],
                                     cvt[cp0:cp1, cq, mr0:mr1],
                                     data[dp0:dp1, dq, 0:320],
                                     start=(t == 0), stop=(t == 2))

        def mm_left(ps, cname, data, dlay, clay, ncols):
            for ch in (0, 1):
                cvt = cv[cname]
                dts = dlay[ch]
                n = len(dts)
                for (mp0, mp1, mq, mr0, mr1) in dlay[ch]:
                    for t in range(n):
                        dp0, dp1, dq, _, _ = dts[t]
                        cp0, cp1, cq, _, _ = clay[t]
                        nc.tensor.matmul(
                            ps[mp0:mp1, mq, 0:ncols],
                            cvt[cp0:cp1, cq, mr0:mr1],
                            data[dp0:dp1, dq, 0:ncols],
                            start=(t == 0), stop=(t == n - 1))

        FWDT = {0: [(0, "ftr"), (1, "ftin")], 1: [(0, "fti"), (1, "ftr")]}
        INVT = {0: [(0, "ifr"), (1, "ifin")], 1: [(0, "ifi"), (1, "ifr")]}

        # ---------- init: x0 = z = iF(y), load consts ----------
        for i in range(IMGS):
            im = per_img[i]
            nc.sync.dma_start(Km[:], dr[f"y{i}"][:])
            nc.sync.dma_start(im["c0"][:], dr[f"c0{i}"][:])
            nc.sync.dma_start(im["mk"][:], dr[f"mk{i}"][:])
            for oc in (0, 1):
                p = psA3()
                mm_fft_ch(p, Km, INVT, oc)
                ew(CP, chv2(sbA, oc), psv(p), pat="av"[oc] + "v")
            for oc in (0, 1):
                p = psA3()
                mm_fft_ch(p, sbA, INVT, oc)
                ew(CP, chv2(im["xA"], oc), psv(p), pat="aa")
                ew(CP, chv2(im["z"], oc), psv(p), pat="vv")

        # ---------- phase generators ----------
        def A_phase(i):
            """FFT + data fidelity: xcb_i = z_i - iF(mk*F(z_i)) + c0_i."""
            im = per_img[i]
            xcb = im["xcb"]
            ew(TT(ALU.add), pc(xcb), pc(im["z"]), pc(im["c0"]), pat="gg")
            yield
            for oc in (0, 1):
                p = psA3()
                mm_fft_ch(p, im["z"], FWDT, oc)
                ew(CP, chv2(sbA, oc), psv(p), pat="av"[oc] + "v")
                yield
            for oc in (0, 1):
                p = psA3()
                mm_fft_ch(p, sbA, FWDT, oc)
                ew(TT(ALU.mult), chv2(Km, oc), psv(p), chv2(im["mk"], oc),
                   pat="vv")
                yield
            for oc in (0, 1):
                p = psA3()
                mm_fft_ch(p, Km, INVT, oc)
                ew(CP, chv2(sbA, oc), psv(p), pat="av"[oc] + "v")
                yield
            for oc in (0, 1):
                p = psA3()
                mm_fft_ch(p, sbA, INVT, oc)
                ew(TT(ALU.subtract), chv2(xcb, oc), chv2(xcb, oc), psv(p),
                   pat="vv")
                yield

        def B_phase(i):
            """TV prox of xcb_i -> xtv (shared; TV runs serialized)."""
            im = per_img[i]
            xcb = im["xcb"]
            if SKIP_TV:
                ew(CP, pc(xtv), pc(xcb), pat="vg")
                yield
                return
            for it in range(TV_ITERS):
                if it == 0:
                    pT = []
                    for ch in (0, 1):
                        p = psA3()
                        mm_tv_ch(p, "dgtf", xcb, True, ch)
                        pT.append(p)
                    ew(TT(ALU.subtract), pc(qx, 2, 321), pc(xcb, 1, 320),
                       pc(xcb, 0, 319), pat="vv")
                    for ch in (0, 1):
                        svs = psv(pT[ch])
                        for j, (sv, dv, cv_) in enumerate(
                                zip(svs, chv2(vt, ch), chv2(qy, ch))):
                            nc.scalar.square(dv, sv)
                            if j == 0:
                                nc.vector.tensor_copy(cv_, sv)
                            else:
                                nc.scalar.copy(cv_, sv)
                    yield
                    ew(SQ, pc(tv1), pc(qx, 2, 322), pat="aa")
                    ew(TT(ALU.add), pc(n2), pc(tv1), pc(vt), pat="vv")
                    ew(SMAX(lam1 * lam1), pc(n2), pc(n2), pat="vv")
                    ew(RSQ(1.0 / (lam * lam)), pc(n2), pc(n2), pat="aa")
                    yield
                    ew(TT(ALU.mult), pc(qx, 2, 321), pc(qx, 2, 321),
                       pc(n2, 0, 319), pat="vv")
                    ew(TT(ALU.mult), pc(qy), pc(qy), pc(n2), pat="vv")
                    yield
                else:
                    ew(TT(ALU.subtract), pc(tv1), pc(xcb), pc(qx, 2, 322),
                       pat="vv")
                    ew(TT(ALU.add), pc(tv1), pc(tv1), pc(qx, 1, 321), pat="vv")
                    yield
                    for ch in (0, 1):
                        p = psA3()
                        mm_tv_ch(p, "ddt", qy, False, ch)
                        ew(TT(ALU.subtract), chv2(vt, ch), chv2(tv1, ch),
                           psv(p), pat="vv")
                    yield
                    pT2 = []
                    for ch in (0, 1):
                        p = psA3()
                        mm_tv_ch(p, "dgt", vt, True, ch)
                        pT2.append(p)
                    ew(TT(ALU.subtract), pc(tv1, 0, 319), pc(vt, 1, 320),
                       pc(vt, 0, 319), pat="vv")
                    ew(STT(s_tv, ALU.mult, ALU.add),
                       pc(qx, 2, 321), pc(tv1, 0, 319), pc(qx, 2, 321), pat="vv")
                    for ch in (0, 1):
                        for sv, dv in zip(psv(pT2[ch]), chv2(qy, ch)):
                            nc.vector.scalar_tensor_tensor(
                                dv, sv, s_tv, dv, ALU.mult, ALU.add)
                    yield
                    ew(SQ, pc(tv1), pc(qx, 2, 322), pat="aa")
                    ew(SQ, pc(vt), pc(qy), pat="av")
                    ew(TT(ALU.add), pc(n2), pc(tv1), pc(vt), pat="vv")
                    ew(SMAX(lam * lam), pc(n2), pc(n2), pat="vv")
                    ew(RSQ(1.0 / (lam * lam)), pc(n2), pc(n2), pat="aa")
                    yield
                    ew(TT(ALU.mult), pc(qx, 2, 321), pc(qx, 2, 321),
                       pc(n2, 0, 319), pat="vv")
                    ew(TT(ALU.mult), pc(qy), pc(qy), pc(n2), pat="vv")
                    yield
            # x_tv = xcb + lam*div(q)/lam  (q carries the lam scaling)
            for ch in (0, 1):
                p = psA3()
                mm_tv_ch(p, "ddt", qy, False, ch)
                for sv, dv, xv in zip(psv(p), chv2(wtmp, ch), chv2(xcb, ch)):
                    nc.vector.scalar_tensor_tensor(dv, sv, -1.0, xv,
                                                   ALU.mult, ALU.add)
            ew(TT(ALU.subtract), pc(xtv), pc(wtmp), pc(qx, 2, 322), pat="vv")
            ew(TT(ALU.add), pc(xtv), pc(xtv), pc(qx, 1, 321), pat="vv")
            yield

        def C_phase(i, k):
            """3-level Haar DWT soft-threshold of xtv -> xnew."""
            im = per_img[i]
            xnew = im["xB"] if k % 2 == 0 else im["xA"]
            if SKIP_DWT:
                ew(CP, pc(xnew), pc(xtv), pat="vg")
                yield
                return
            ew(TT(ALU.add), pc(wdwt, 0, 160), pc(xtv, 0, 320, 2),
               pc(xtv, 1, 320, 2), pat="vg")
            ew(TT(ALU.subtract), pc(wdwt, 160, 320), pc(xtv, 0, 320, 2),
               pc(xtv, 1, 320, 2), pat="gg")
            yield
            for ch in (0, 1):
                p = psA3()
                mm_w1_ch(p, "w1t", wdwt, ch)
                ew(CP, chv2(Y1, ch), psv(p), pat="av"[ch] + "v")
                yield
            segeng = os.environ.get("CS_SEG", "vgg")
            for ch in (0, 1):
                for si, (sp, sq, dp, dq, cnt) in enumerate(SEG12[ch]):
                    e = ENG[segeng[(ch + si) % 3]]
                    e.tensor_tensor(L2t[dp:dp + cnt, dq, 0:80],
                                    Y1[sp:sp + cnt, sq, 0:160:2],
                                    Y1[sp:sp + cnt, sq, 1:160:2], ALU.add)
                    e.tensor_tensor(L2t[dp:dp + cnt, dq, 80:160],
                                    Y1[sp:sp + cnt, sq, 0:160:2],
                                    Y1[sp:sp + cnt, sq, 1:160:2], ALU.subtract)
            yield
            for ch in (0, 1):
                pY2 = psB()
                mm_lvl(pY2, "w2t", L2t, L2D, L2C, 160, ch)
                nc.scalar.copy(Y2[0:128, 2 * ch, :], pY2[0:128, 0, 0:160])
                nc.scalar.copy(Y2[0:32, 2 * ch + 1, :], pY2[0:32, 1, 0:160])
            for ch in (0, 1):
                for si, (sp, sq, dp, dq, cnt) in enumerate(SEG23[ch]):
                    e = ENG[segeng[(ch + si) % 3]]
                    e.tensor_tensor(L3t[dp:dp + cnt, dq, 0:40],
                                    Y2[sp:sp + cnt, sq, 0:80:2],
                                    Y2[sp:sp + cnt, sq, 1:80:2], ALU.add)
                    e.tensor_tensor(L3t[dp:dp + cnt, dq, 40:80],
                                    Y2[sp:sp + cnt, sq, 0:80:2],
                                    Y2[sp:sp + cnt, sq, 1:80:2], ALU.subtract)
            yield
            for ch in (0, 1):
                pY3 = psB()
                mm_lvl(pY3, "w3t", L3t, L3D, L3C, 80, ch)
                nc.scalar.copy(Y3[0:64, 2 * ch, :], pY3[0:64, 0, 0:80])
                nc.scalar.copy(Y3[0:16, 2 * ch + 1, :], pY3[0:16, 1, 0:80])
            # thresholds: save ll3, soft-threshold everything, restore ll3
            nc.scalar.copy(L3t[0:40, 0:4:2, 0:40], Y3[0:40, 0:4:2, 0:40])
            soft_views([Y3[0:64, 0:4:2, :], Y3[0:16, 1:4:2, :]], lam_lvl[2],
                       [wdwt[0:64, 0:4:2, 0:80], wdwt[0:16, 1:4:2, 0:80]],
                       pat="vg")
            nc.scalar.copy(Y3[0:40, 0:4:2, 0:40], L3t[0:40, 0:4:2, 0:40])
            soft_views([Y2[0:128, 0:4:2, :], Y2[0:32, 1:4:2, :]], lam_lvl[1],
                       [wdwt[0:128, 0:4:2, 0:160], wdwt[0:32, 1:4:2, 0:160]],
                       pat="gv")
            yield
            soft_views(pc(Y1), lam_lvl[0], pc(wdwt), pat="gg")
            yield
            # ---------- inverse ----------
            for ch in (0, 1):
                pZ3 = psB()
                mm_lvl(pZ3, "w3h", Y3, L3D, L3C, 80, ch)
                nc.scalar.copy(L3t[0:64, 2 * ch, :], pZ3[0:64, 0, 0:80])
                nc.scalar.copy(L3t[0:16, 2 * ch + 1, :], pZ3[0:16, 1, 0:80])
            for ch in (0, 1):
                for si, (sp, sq, dp, dq, cnt) in enumerate(SEG23[ch]):
                    e = ENG[segeng[(ch + si) % 3]]
                    e.tensor_tensor(Y2[sp:sp + cnt, sq, 0:80:2],
                                    L3t[dp:dp + cnt, dq, 0:40],
                                    L3t[dp:dp + cnt, dq, 40:80], ALU.add)
                    e.tensor_tensor(Y2[sp:sp + cnt, sq, 1:80:2],
                                    L3t[dp:dp + cnt, dq, 0:40],
                                    L3t[dp:dp + cnt, dq, 40:80], ALU.subtract)
            yield
            for ch in (0, 1):
                pZ2 = psB()
                mm_lvl(pZ2, "w2h", Y2, L2D, L2C, 160, ch)
                nc.scalar.copy(L2t[0:128, 2 * ch, :], pZ2[0:128, 0, 0:160])
                nc.scalar.copy(L2t[0:32, 2 * ch + 1, :], pZ2[0:32, 1, 0:160])
            for ch in (0, 1):
                for si, (sp, sq, dp, dq, cnt) in enumerate(SEG12[ch]):
                    e = ENG[segeng[(ch + si) % 3]]
                    e.tensor_tensor(Y1[sp:sp + cnt, sq, 0:160:2],
                                    L2t[dp:dp + cnt, dq, 0:80],
                                    L2t[dp:dp + cnt, dq, 80:160], ALU.add)
                    e.tensor_tensor(Y1[sp:sp + cnt, sq, 1:160:2],
                                    L2t[dp:dp + cnt, dq, 0:80],
                                    L2t[dp:dp + cnt, dq, 80:160], ALU.subtract)
            yield
            for ch in (0, 1):
                p = psA3()
                mm_w1_ch(p, "w1h", Y1, ch)
                ew(CP, chv2(wdwt, ch), psv(p), pat="av"[ch] + "v")
                yield
            ew(TT(ALU.add), pc(xnew, 0, 320, 2), pc(wdwt, 0, 160),
               pc(wdwt, 160, 320), pat="gg")
            ew(TT(ALU.subtract), pc(xnew, 1, 320, 2), pc(wdwt, 0, 160),
               pc(wdwt, 160, 320), pat="vg")
            yield

        def D_phase(i, k):
            """FISTA momentum: z = xnew + c_k (xnew - xold)."""
            im = per_img[i]
            xold = im["xA"] if k % 2 == 0 else im["xB"]
            xnew = im["xB"] if k % 2 == 0 else im["xA"]
            if k < MAX_ITER - 1:
                ew(TT(ALU.subtract), pc(Y1), pc(xnew), pc(xold), pat="gg")
                ew(STT(coeffs[k], ALU.mult, ALU.add),
                   pc(im["z"]), pc(Y1), pc(xnew), pat="vv")
            yield

        def BCD(i, k):
            yield from B_phase(i)
            yield from C_phase(i, k)
            yield from D_phase(i, k)

        def CD(i, k):
            yield from C_phase(i, k)
            yield from D_phase(i, k)

        def interleave(g1, g2, ratio, lead=2):
            """Drain g1 fully, advancing g2 once per `ratio` g1 chunks."""
            live2 = g2 is not None
            for _ in range(lead if live2 else 0):
                try:
                    next(g2)
                except StopIteration:
                    live2 = False
                    break
            n = 0
            for _ in g1:
                n += 1
                if live2 and n % ratio == 0:
                    try:
                        next(g2)
                    except StopIteration:
                        live2 = False
            if live2:
                for _ in g2:
                    pass

        # ---------- FISTA: 2-image software pipeline ----------
        # slot(0,k): BCD(img0, k) || A(img1, k)
        # slot(1,k): BCD(img1, k) || A(img0, k+1)
        for _ in A_phase(0):
            pass
        THREEWAY = os.environ.get("CS_3WAY", "1") == "1"
        for k in range(MAX_ITER):
            if THREEWAY:
                R1 = int(os.environ.get("CS_R1", "3"))
                R2 = int(os.environ.get("CS_R2", "1"))
                R3 = int(os.environ.get("CS_R3", "1"))
                interleave(B_phase(0), A_phase(1), ratio=R1, lead=int(os.environ.get("CS_L1", "2")))
                interleave(B_phase(1), CD(0, k), ratio=R2, lead=int(os.environ.get("CS_L2", "1")))
                nxt = A_phase(0) if k < MAX_ITER - 1 else None
                interleave(CD(1, k), nxt, ratio=R3, lead=int(os.environ.get("CS_L3", "1")))
            else:
                interleave(BCD(0, k), A_phase(1), ratio=3, lead=2)
                nxt = A_phase(0) if k < MAX_ITER - 1 else None
                interleave(BCD(1, k), nxt, ratio=3, lead=2)

        fin = "xB" if (MAX_ITER - 1) % 2 == 0 else "xA"
        for i in range(IMGS):
            nc.sync.dma_start(dr[f"xo{i}"][:], per_img[i][fin][:])

    nc.compile()
    return nc


_NC = None


def _get_nc():
    global _NC
    if _NC is None:
        _NC = _build_nc()
    return _NC


def kernel(y, mask):
    from concourse.bass_utils import run_bass_kernel_spmd

    y = np.asarray(y, dtype=np.float32)
    mask = np.asarray(mask, dtype=np.float32)
    nc = _get_nc()
    in_maps = build_in_maps(y, mask)

    res = run_bass_kernel_spmd(nc, in_maps, list(range(NCORES)))
    global LAST_RES
    LAST_RES = res
    out = np.zeros((B, 2, H, W), dtype=np.float32)
    for core in range(NCORES):
        for i in range(IMGS):
            out[core * IMGS + i] = _unpack_p6(res.results[core][f"xo{i}"])
    return out



# revision 39
# speedup vs baseline: 1.3086x; 1.3086x over previous
"""Trainium2 Bass kernel for nn_CombinedCS (FISTA compressed-sensing recon).

Self-contained: hardcodes shapes (B=16, H=W=320), shards batch over 8 cores
(2 images per core), runs the full 15-iteration FISTA loop SBUF-resident.

Math plan (validated vs reference):
  - centered 2D FFT as two PE matmul stages against the DFT matrix F
    (transpose-free: data is always lhsT, F^T always rhs)
  - data fidelity uses the binary mask identity m*(m*F z - y) = m*F z - m*y,
    so z_step = z - iF(m*F z) + c0 with c0 = iF(m*y) precomputed on host
  - TV prox (5 Chambolle iters): h-direction div/grad as PE left-mults by
    BANDED bidiagonal matrices (only the 2 contraction tiles that carry the
    band); w-direction via shifted free-dim views with zero guard columns;
    inner loop in bf16 (2x DVE rate), duals carry a lam scaling
  - 3-level Haar DWT: w-step unnormalized (a+b, a-b) on DVE/Pool, h-step as
    PE left-mult by orthonormal Haar matrix; detail soft-threshold via
    x - clamp(x, -t, t); inverse folds the w-step 1/2 into the h-step matrix

Layout P6: one complex image (2 ch x 320 x 320) packs into
[128 partitions, 6 blocks, 320]; channel ch occupies blocks 3ch..3ch+2
with h = 128*qb + p (block 3ch+2 uses p<64; its p>=64 "dead" region is kept
zero/finite so ops can process whole channels as single [128,3,w] views).

Scheduling: the two images per core are software-pipelined — image (i+1)'s
PE-heavy FFT phase is interleaved chunk-by-chunk with image i's DVE-heavy
TV/DWT/momentum phases. PSUM: pool psa = [128,3,512] x 2 bufs (6 banks, all
per-channel matmul groups), psb = [128,4,256] (2 banks, DWT level 2/3).
Elementwise work is spread across DVE ("v"), Pool ("g", TensorTensor/copy
only, no PSUM), and Act ("a", activations/copies).
"""
import math
import os

import ml_dtypes
import numpy as np

H = W = 320
B = 16
NCORES = 8
IMGS = B // NCORES  # 2
LAM_TV = 0.005
LAM_WAV = 0.005
TAU = 0.25
TV_ITERS = 5
LEVELS = 3
MAX_ITER = int(os.environ.get("CS_ITERS", "15"))
SKIP_TV = os.environ.get("CS_SKIP_TV", "0") == "1"
SKIP_DWT = os.environ.get("CS_SKIP_DWT", "0") == "1"
S2 = math.sqrt(2.0)

# layouts: per ch, list of (p0, p1, q, r0, r1): matrix rows r0..r1 live at
# partitions p0..p1 of block q. All tiles base-0 (matmul dst requirement).
P6D = {
    0: [(0, 128, 0, 0, 128), (0, 128, 1, 128, 256), (0, 64, 2, 256, 320)],
    1: [(0, 128, 3, 0, 128), (0, 128, 4, 128, 256), (0, 64, 5, 256, 320)],
}
P6C = [(0, 128, 0, 0, 128), (0, 128, 1, 128, 256), (0, 64, 2, 256, 320)]
L2D = {
    0: [(0, 128, 0, 0, 128), (0, 32, 1, 128, 160)],
    1: [(0, 128, 2, 0, 128), (0, 32, 3, 128, 160)],
}
L2C = [(0, 128, 0, 0, 128), (0, 32, 1, 128, 160)]
L3D = {
    0: [(0, 64, 0, 0, 64), (0, 16, 1, 64, 80)],
    1: [(0, 64, 2, 0, 64), (0, 16, 3, 64, 80)],
}
L3C = [(0, 64, 0, 0, 64), (0, 16, 1, 64, 80)]


def _dft_mats():
    I = np.eye(H, dtype=np.complex128)
    F = np.fft.fftshift(
        np.fft.fft(np.fft.ifftshift(I, axes=0), axis=0, norm="ortho"), axes=0
    )
    G = np.conj(F).T
    return F, G


def _tv_mats():
    Dd = np.zeros((H, H))
    Dd[0, 0] = 1.0
    for h in range(1, H - 1):
        Dd[h, h] = 1.0
        Dd[h, h - 1] = -1.0
    Dd[H - 1, H - 2] = -1.0
    Dg = np.zeros((H, H))
    for h in range(H - 1):
        Dg[h, h] = -1.0
        Dg[h, h + 1] = 1.0
    return Dd, Dg


def _haar_mat(n):
    Wm = np.zeros((n, n))
    hn = n // 2
    c = 1.0 / S2
    for i in range(hn):
        Wm[i, 2 * i] = c
        Wm[i, 2 * i + 1] = c
        Wm[hn + i, 2 * i] = c
        Wm[hn + i, 2 * i + 1] = -c
    return Wm


def _momentum_coeffs():
    t = 1.0
    out = []
    for _ in range(MAX_ITER):
        t_new = (1.0 + math.sqrt(1.0 + 4.0 * t * t)) / 2.0
        out.append((t - 1.0) / t_new)
        t = t_new
    return out


def _pack_p6(x):
    """x: (2, 320, 320) -> (128, 6, 320), zero-padded dead region."""
    out = np.zeros((128, 6, 320), dtype=x.dtype)
    for ch in range(2):
        out[:, 3 * ch + 0] = x[ch, 0:128]
        out[:, 3 * ch + 1] = x[ch, 128:256]
        out[0:64, 3 * ch + 2] = x[ch, 256:320]
    return out


def _unpack_p6(p):
    out = np.zeros((2, 320, 320), dtype=p.dtype)
    for ch in range(2):
        out[ch, 0:128] = p[:, 3 * ch + 0]
        out[ch, 128:256] = p[:, 3 * ch + 1]
        out[ch, 256:320] = p[0:64, 3 * ch + 2]
    return out


def _host_consts():
    F, G = _dft_mats()
    Dd, Dg = _tv_mats()
    W1, W2, W3 = _haar_mat(320), _haar_mat(160), _haar_mat(80)
    f32 = np.float32
    bf16 = ml_dtypes.bfloat16
    return {
        "ftr": F.real.T.astype(f32), "fti": F.imag.T.astype(f32),
        "ftin": (-F.imag.T).astype(f32),
        "ifr": G.real.T.astype(f32), "ifi": G.imag.T.astype(f32),
        "ifin": (-G.imag.T).astype(f32),
        "ddt": Dd.T.astype(bf16), "dgt": Dg.T.astype(bf16),
        "idn": (-np.eye(128)).astype(bf16), "idp": np.eye(128).astype(bf16),
        "idnf": (-np.eye(128)).astype(f32),
        "w1t": W1.T.astype(f32), "w1h": (0.5 * W1).astype(f32),
        "w2t": W2.T.astype(f32), "w2h": (0.5 * W2).astype(f32),
        "w3t": W3.T.astype(f32), "w3h": (0.5 * W3).astype(f32),
    }


def _ifft2c_np(x):
    # x: (2, H, W) real/imag -> centered inverse 2D FFT, same layout
    xc = x[0] + 1j * x[1]
    ic = np.fft.fftshift(
        np.fft.ifft2(np.fft.ifftshift(xc, axes=(-2, -1)), norm="ortho"),
        axes=(-2, -1))
    return np.stack([ic.real, ic.imag], axis=0).astype(np.float32)


def build_in_maps(y, mask):
    """Per-core input maps. c0_i = iF(mask*y) exploits the binary mask:
    mask*(mask*F(z) - y) = mask*F(z) - mask*y, so the data-fidelity step is
    z - iF(mask*F(z)) + c0 with c0 constant across iterations."""
    c = _host_consts()
    in_maps = []
    for core in range(NCORES):
        m = dict(c)
        m["zz"] = np.zeros((128, 6, 322), dtype=ml_dtypes.bfloat16)
        m["zzf"] = np.zeros((128, 6, 320), dtype=np.float32)
        for i in range(IMGS):
            b = core * IMGS + i
            mpair = np.broadcast_to(mask[b], (2, 320, 320)).astype(np.float32)
            m[f"y{i}"] = _pack_p6(y[b])
            m[f"c0{i}"] = _pack_p6(_ifft2c_np(mask[b] * y[b]))
            m[f"mk{i}"] = _pack_p6(mpair).astype(ml_dtypes.bfloat16)
        in_maps.append(m)
    return in_maps


def _copy_segs(src_lay, dst_lay, nrows):
    out = {}
    for ch in (0, 1):
        def locate(lay, r):
            for (p0, p1, q, r0, r1) in lay[ch]:
                if r0 <= r < r1:
                    return p0 + (r - r0), q, r1 - r
            raise AssertionError(r)
        segs = []
        r = 0
        while r < nrows:
            sp, sq, sleft = locate(src_lay, r)
            dp, dq, dleft = locate(dst_lay, r)
            cnt = min(sleft, dleft, nrows - r)
            segs.append((sp, sq, dp, dq, cnt))
            r += cnt
        out[ch] = segs
    return out


SEG12 = _copy_segs(P6D, L2D, 160)
SEG23 = _copy_segs(L2D, L3D, 80)


def _build_nc():
    import concourse.bacc as bacc
    import concourse.tile as tile
    import concourse.mybir as mybir
    from contextlib import ExitStack

    dt = mybir.dt
    F32, F32R, BF16 = dt.float32, dt.float32r, dt.bfloat16
    ALU = mybir.AluOpType
    AF = mybir.ActivationFunctionType

    s_tv = TAU * LAM_TV
    lam = LAM_TV
    eps_q = lam * lam * 1e-8
    lam1 = lam / s_tv
    eps1 = eps_q / (s_tv * s_tv)
    coeffs = _momentum_coeffs()
    lam_lvl = [LAM_WAV * (S2 ** (l + 1)) for l in range(LEVELS)]

    nc = bacc.Bacc("TRN2", target_bir_lowering=False, debug=False,
                   num_devices=NCORES)

    dr = {}
    for name in ("ftr", "fti", "ftin", "ifr", "ifi", "ifin", "w1t", "w1h"):
        dr[name] = nc.dram_tensor(name, [320, 320], F32R, kind="ExternalInput").ap()
    for name in ("w2t", "w2h"):
        dr[name] = nc.dram_tensor(name, [160, 160], F32R, kind="ExternalInput").ap()
    for name in ("w3t", "w3h"):
        dr[name] = nc.dram_tensor(name, [80, 80], F32R, kind="ExternalInput").ap()
    for name in ("ddt", "dgt"):
        dr[name] = nc.dram_tensor(name, [320, 320], BF16, kind="ExternalInput").ap()
    for name in ("idn", "idp"):
        dr[name] = nc.dram_tensor(name, [128, 128], BF16, kind="ExternalInput").ap()
    dr["idnf"] = nc.dram_tensor("idnf", [128, 128], F32R, kind="ExternalInput").ap()
    dr["zz"] = nc.dram_tensor("zz", [128, 6, 322], BF16, kind="ExternalInput").ap()
    dr["zzf"] = nc.dram_tensor("zzf", [128, 6, 320], F32R, kind="ExternalInput").ap()
    for i in range(IMGS):
        dr[f"y{i}"] = nc.dram_tensor(f"y{i}", [128, 6, 320], F32R, kind="ExternalInput").ap()
        dr[f"c0{i}"] = nc.dram_tensor(f"c0{i}", [128, 6, 320], F32, kind="ExternalInput").ap()
        dr[f"mk{i}"] = nc.dram_tensor(f"mk{i}", [128, 6, 320], BF16, kind="ExternalInput").ap()
        dr[f"xo{i}"] = nc.dram_tensor(f"xo{i}", [128, 6, 320], F32, kind="ExternalOutput").ap()

    with ExitStack() as ctx:
        tc = ctx.enter_context(tile.TileContext(nc))
        st = ctx.enter_context(tc.tile_pool(name="state", bufs=1))
        # 1-bank instances, depth 6: lets PE run several matmul groups ahead
        # of the (DVE/Act) PSUM consumers instead of stalling at depth 2
        psa = ctx.enter_context(tc.tile_pool(name="psa", bufs=6, space="PSUM"))
        psb = ctx.enter_context(tc.tile_pool(name="psb", bufs=2, space="PSUM"))

        def T(tag, shape, dtp):
            return st.tile(shape, dtp, tag=tag, name=tag)

        cv = {}
        for name in ("ftr", "fti", "ftin", "ifr", "ifi", "ifin", "w1t", "w1h"):
            cv[name] = T("c_" + name, [128, 3, 320], F32R)
        for name in ("ddt", "dgt"):
            cv[name] = T("c_" + name, [128, 3, 320], BF16)
        for name in ("idn", "idp"):
            cv[name] = T("c_" + name, [128, 1, 128], BF16)
        cv["idnf"] = T("c_idnf", [128, 1, 128], F32R)
        for name in ("w2t", "w2h"):
            cv[name] = T("c_" + name, [128, 2, 160], F32R)
        for name in ("w3t", "w3h"):
            cv[name] = T("c_" + name, [128, 2, 80], F32R)

        def load_const(name, lay):
            for (p0, p1, q, r0, r1) in lay:
                nc.sync.dma_start(cv[name][p0:p1, q, :], dr[name][r0:r1, :])

        for name in ("ftr", "fti", "ftin", "ifr", "ifi", "ifin", "w1t", "w1h",
                     "ddt", "dgt"):
            load_const(name, P6C)
        for name in ("w2t", "w2h"):
            load_const(name, L2C)
        for name in ("w3t", "w3h"):
            load_const(name, L3C)
        for name in ("idn", "idp", "idnf"):
            nc.sync.dma_start(cv[name][0:128, 0, :], dr[name][0:128, :])

        per_img = []
        for i in range(IMGS):
            per_img.append({
                "z": T(f"z{i}", [128, 6, 320], F32R),
                "xA": T(f"xA{i}", [128, 6, 320], F32),
                "xB": T(f"xB{i}", [128, 6, 320], F32),
                "c0": T(f"c0k{i}", [128, 6, 320], F32),
                "mk": T(f"msk{i}", [128, 6, 320], BF16),
                "xcb": T(f"xcb{i}", [128, 6, 320], F32R),
                # per-image TV state so B(0) and B(1) can overlap
                "qx": T(f"qx{i}", [128, 6, 322], BF16),
                "qy": T(f"qy{i}", [128, 6, 320], BF16),
                "sv": T(f"sv{i}", [128, 6, 320], BF16),
                "tv1": T(f"tv1{i}", [128, 6, 320], BF16),
                "n2": T(f"n2{i}", [128, 6, 320], BF16),
            })
        sbA = T("sbA", [128, 6, 320], F32R)
        Km = T("Km", [128, 6, 320], F32R)
        xtv = T("xtv", [128, 6, 320], F32R)
        Y1 = T("Y1", [128, 6, 320], F32R)
        Y2 = T("Y2", [128, 4, 160], F32R)
        Y3 = T("Y3", [128, 4, 80], F32R)
        L2t = T("L2t", [128, 4, 160], F32R)
        L3t = T("L3t", [128, 4, 80], F32R)
        wdwt = T("wdwt", [128, 6, 320], F32R)

        def psA1():
            # one PSUM bank per out tile (320 of 512 f32 used)
            return psa.tile([128, 1, 512], F32, tag="A", name="psA")

        def psB():
            # per-channel lvl2/3 instance: 2 blocks x 256 f32 = 1 bank
            return psb.tile([128, 2, 256], F32, tag="B", name="psB")

        def mm_lvl(ps, cname, data, dlay, clay, ncols, ch):
            # per-channel level-2/3 left-mult; out blocks remapped to 0,1
            cvt = cv[cname]
            dts = dlay[ch]
            n = len(dts)
            for mi, (mp0, mp1, mq, mr0, mr1) in enumerate(dts):
                for t in range(n):
                    dp0, dp1, dq, _, _ = dts[t]
                    cp0, cp1, cq, _, _ = clay[t]
                    nc.tensor.matmul(
                        ps[mp0:mp1, mi, 0:ncols],
                        cvt[cp0:cp1, cq, mr0:mr1],
                        data[dp0:dp1, dq, 0:ncols],
                        start=(t == 0), stop=(t == n - 1))

        # zero-init via DMA (memset is not ISA-legal for f32r/bf16 here);
        # covers guard columns of qx and dead regions read via whole-channel
        # views (sv, xtv, Y1, z)
        nc.sync.dma_start(Y1[:], dr["zzf"][:])
        nc.sync.dma_start(xtv[:], dr["zzf"][:])
        for i in range(IMGS):
            im = per_img[i]
            nc.sync.dma_start(im["qx"][:], dr["zz"][:])
            nc.sync.dma_start(im["qy"][:], dr["zz"][0:128, 0:6, 0:320])
            nc.sync.dma_start(im["sv"][:], dr["zz"][0:128, 0:6, 0:320])
            nc.sync.dma_start(im["z"][:], dr["zzf"][:])
            nc.vector.memset(im["xA"][:], 0.0)

        # ----- whole-channel views: [128, 3 blocks, w] incl. dead region -----
        def chv(t, ch, c0=0, c1=None, step=1):
            c1 = t.shape[-1] if c1 is None else c1
            if step != 1:
                return t[0:128, 3 * ch:3 * ch + 3, c0:c1:step]
            return t[0:128, 3 * ch:3 * ch + 3, c0:c1]

        def pc(t, c0=0, c1=None, step=1):
            return [chv(t, 0, c0, c1, step), chv(t, 1, c0, c1, step)]

        def psv(p, c0=0, c1=320):
            # valid sub-views only: block-2 partitions 64:128 are never
            # written by matmuls (stale bytes from the prior pool instance)
            return [p[0:128, 0:2, c0:c1], p[0:64, 2, c0:c1]]

        def chv2(t, ch, c0=0, c1=None, step=1):
            c1 = t.shape[-1] if c1 is None else c1
            b = 3 * ch
            if step != 1:
                return [t[0:128, b:b + 2, c0:c1:step],
                        t[0:64, b + 2, c0:c1:step]]
            return [t[0:128, b:b + 2, c0:c1], t[0:64, b + 2, c0:c1]]

        NO_POOL = os.environ.get("CS_NO_POOL", "0") == "1"
        ENG = {"v": nc.vector,
               "g": nc.vector if NO_POOL else nc.gpsimd,
               "a": nc.scalar}

        # ew: dispatch per-channel ops across DVE ("v") / Pool ("g") / Act
        # ("a") per 2-char pattern. Pool only supports plain TensorTensor /
        # TensorCopy (no PSUM), so other ops fall back to DVE on "g".
        def ew(fn, *views, pat="vv"):
            for i in range(len(views[0])):
                e = ENG[pat[i % len(pat)]]
                if e is nc.gpsimd and not getattr(fn, "pool_ok", False):
                    e = nc.vector
                fn(e, *[v[i] for v in views])

        def TT(op):
            fn = lambda e, o, a, b: e.tensor_tensor(o, a, b, op)
            fn.pool_ok = True
            return fn

        def STT(s, op0, op1):
            return lambda e, o, a, b: e.scalar_tensor_tensor(o, a, s, b, op0, op1)

        def SMAX(s):
            return lambda e, o, a: e.tensor_scalar_max(o, a, s)

        def SQ(e, o, a):
            if e is nc.scalar:
                e.square(o, a)
            else:
                e.tensor_tensor(o, a, a, ALU.mult)

        def CP(e, o, a):
            if e is nc.scalar:
                e.copy(o, a)
            else:
                e.tensor_copy(o, a)
        CP.pool_ok = True

        def RSQ(scale):
            return lambda e, o, a: nc.scalar.activation(
                o, a, AF.Abs_reciprocal_sqrt, scale=scale)

        def soft_views(views, lam_l, tmp_views, pat="vg"):
            # soft(x, t) = x - clamp(x, -t, t); clamp is DVE-only (Pool has
            # no TensorScalar), the subtract may go to Pool
            for i in range(len(views)):
                e = ENG[pat[i % len(pat)]]
                nc.vector.tensor_scalar(tmp_views[i], views[i], -lam_l, lam_l,
                                        ALU.max, ALU.min)
                te = e if e is not nc.scalar else nc.vector
                te.tensor_tensor(views[i], views[i], tmp_views[i], ALU.subtract)

        # ------- per-out-tile matmul stages (one PSUM bank per group) -------
        # consume(ti, pv, mp0, mp1): pv is the [cnt, 320] PSUM view of out
        # tile ti; dst block for channel c is 3*c + ti.
        def stage_fft(data, terms, oc, consume):
            for ti, (mp0, mp1, mq, mr0, mr1) in enumerate(P6C):
                p = psA1()
                mml = []
                for (dch, cname) in terms[oc]:
                    cvt = cv[cname]
                    for t in range(3):
                        dp0, dp1, dq, _, _ = P6D[dch][t]
                        cp0, cp1, cq, _, _ = P6C[t]
                        mml.append((data[dp0:dp1, dq, mr0:mr1],
                                    cvt[cp0:cp1, cq, 0:320]))
                for idx, (l, r) in enumerate(mml):
                    nc.tensor.matmul(p[mp0:mp1, 0, 0:320], l, r,
                                     start=(idx == 0), stop=(idx == len(mml) - 1))
                consume(ti, p[mp0:mp1, 0, 0:320], mp0, mp1)

        def stage_tv(cname, data, upper, ch, consume, fold=None):
            """Banded bidiagonal group per out tile ({ti-1,ti} lower / {ti,ti+1}
            upper), optionally with identity terms fold=[(idname, tensor, c0)]
            prepended (column offset c0 realizes free-dim shifts for free)."""
            cvt = cv[cname]
            dts = P6D[ch]
            for ti in range(3):
                mp0, mp1, mq, mr0, mr1 = P6C[ti]
                cnt = mp1 - mp0
                dp0, dp1, dq, _, _ = dts[ti]
                mml = []
                for (idname, tsr, c0) in (fold or []):
                    mml.append((cv[idname][0:cnt, 0, 0:cnt],
                                tsr[dp0:dp1, dq, c0:c0 + 320]))
                need = [ti - 1, ti] if not upper else [ti, ti + 1]
                for t in need:
                    if t < 0 or t > 2:
                        continue
                    cp0, cp1, cq, _, _ = P6C[t]
                    bdp0, bdp1, bdq, _, _ = dts[t]
                    mml.append((cvt[cp0:cp1, cq, mr0:mr1],
                                data[bdp0:bdp1, bdq, 0:320]))
                p = psA1()
                for idx, (l, r) in enumerate(mml):
                    nc.tensor.matmul(p[mp0:mp1, 0, 0:320], l, r,
                                     start=(idx == 0), stop=(idx == len(mml) - 1))
                consume(ti, p[mp0:mp1, 0, 0:320], mp0, mp1)

        def stage_w1(cname, data, ch, consume):
            cvt = cv[cname]
            dts = P6D[ch]
            for ti in range(3):
                mp0, mp1, mq, mr0, mr1 = P6C[ti]
                p = psA1()
                for t in range(3):
                    dp0, dp1, dq, _, _ = dts[t]
                    cp0, cp1, cq, _, _ = P6C[t]
                    nc.tensor.matmul(p[mp0:mp1, 0, 0:320],
                                     cvt[cp0:cp1, cq, mr0:mr1],
                                     data[dp0:dp1, dq, 0:320],
                                     start=(t == 0), stop=(t == 2))
                consume(ti, p[mp0:mp1, 0, 0:320], mp0, mp1)

        def F6(t, c0=0, c1=None, step=1):
            """Whole-tensor view: both channels' blocks in one op."""
            c1 = t.shape[-1] if c1 is None else c1
            if step != 1:
                return t[0:128, 0:6, c0:c1:step]
            return t[0:128, 0:6, c0:c1]

        def cp_pv(dst, ch, pat="vav"):
            """consume: copy PSUM tile into dst's channel-ch block."""
            def consume(ti, pv, mp0, mp1):
                dv = dst[mp0:mp1, 3 * ch + ti, 0:320]
                if pat[ti] == "a":
                    nc.scalar.copy(dv, pv)
                else:
                    nc.vector.tensor_copy(dv, pv)
            return consume

        def scale_pv(dst, ch, s, pat="ava"):
            """consume: dst block = s * PSUM tile."""
            def consume(ti, pv, mp0, mp1):
                dv = dst[mp0:mp1, 3 * ch + ti, 0:320]
                if pat[ti] == "a":
                    nc.scalar.activation(dv, pv, AF.Copy, scale=s)
                else:
                    nc.vector.tensor_scalar_mul(dv, pv, s)
            return consume

        def tt_pv(dst, ch, other, op, rev=False):
            """consume: dst block = other_block op pv (DVE; rev swaps)."""
            def consume(ti, pv, mp0, mp1):
                dv = dst[mp0:mp1, 3 * ch + ti, 0:320]
                ov = other[mp0:mp1, 3 * ch + ti, 0:320]
                if rev:
                    nc.vector.tensor_tensor(dv, pv, ov, op)
                else:
                    nc.vector.tensor_tensor(dv, ov, pv, op)
            return consume

        def mm_left(ps, cname, data, dlay, clay, ncols):
            for ch in (0, 1):
                cvt = cv[cname]
                dts = dlay[ch]
                n = len(dts)
                for (mp0, mp1, mq, mr0, mr1) in dlay[ch]:
                    for t in range(n):
                        dp0, dp1, dq, _, _ = dts[t]
                        cp0, cp1, cq, _, _ = clay[t]
                        nc.tensor.matmul(
                            ps[mp0:mp1, mq, 0:ncols],
                            cvt[cp0:cp1, cq, mr0:mr1],
                            data[dp0:dp1, dq, 0:ncols],
                            start=(t == 0), stop=(t == n - 1))

        FWDT = {0: [(0, "ftr"), (1, "ftin")], 1: [(0, "fti"), (1, "ftr")]}
        INVT = {0: [(0, "ifr"), (1, "ifin")], 1: [(0, "ifi"), (1, "ifr")]}

        # ---------- init: x0 = z = iF(y), load consts ----------
        for i in range(IMGS):
            im = per_img[i]
            nc.sync.dma_start(Km[:], dr[f"y{i}"][:])
            nc.sync.dma_start(im["c0"][:], dr[f"c0{i}"][:])
            nc.sync.dma_start(im["mk"][:], dr[f"mk{i}"][:])
            for oc in (0, 1):
                stage_fft(Km, INVT, oc, cp_pv(sbA, oc, pat="vav"))
            for oc in (0, 1):
                def both(ti, pv, mp0, mp1, oc=oc, im=im):
                    nc.scalar.copy(im["xA"][mp0:mp1, 3 * oc + ti, 0:320], pv)
                    nc.vector.tensor_copy(im["z"][mp0:mp1, 3 * oc + ti, 0:320],
                                          pv)
                stage_fft(sbA, INVT, oc, both)

        # ---------- phase generators ----------
        def A_phase(i):
            """FFT + data fidelity: xcb_i = z_i - iF(mk*F(z_i)) + c0_i."""
            im = per_img[i]
            xcb = im["xcb"]
            ew(TT(ALU.add), pc(xcb), pc(im["z"]), pc(im["c0"]), pat="gg")
            yield 800
            for oc in (0, 1):
                stage_fft(im["z"], FWDT, oc, cp_pv(sbA, oc, pat="ava"))
                yield 2400
            for oc in (0, 1):
                stage_fft(sbA, FWDT, oc, tt_pv(Km, oc, im["mk"], ALU.mult))
                yield 2400
            for oc in (0, 1):
                stage_fft(Km, INVT, oc, cp_pv(sbA, oc, pat="ava"))
                yield 2400
            for oc in (0, 1):
                stage_fft(sbA, INVT, oc, tt_pv(xcb, oc, xcb, ALU.subtract))
                yield 2400

        def B_phase(i):
            """TV prox of xcb_i -> xtv (shared; TV runs serialized).

            All-bf16 inner loop; the two PSUM-consuming updates are computed
            as identity-matmul folds (psA = ±I*a + band*b) so the only PSUM
            readers are Act scaled-copies. sv = s_tv*(tv1 - div_h(qy)) is the
            scaled Chambolle w; qy += Dg*sv and qx += grad_w(sv) then need no
            scalar_tensor_tensor (which gets no DVE fast mode)."""
            im = per_img[i]
            xcb = im["xcb"]
            qx, qy = im["qx"], im["qy"]
            sv, tv1, n2 = im["sv"], im["tv1"], im["n2"]
            if SKIP_TV:
                ew(CP, pc(xtv), pc(xcb), pat="vg")
                yield
                return
            # identity terms realizing  -(xcb - div_w(qx))  inside the Dd
            # group:  -xcb + qx[w] - qx[w-1]  (guard cols make bounds exact)
            W_FOLD = [("idnf", xcb, 0), ("idp", qx, 2), ("idn", qx, 1)]
            for it in range(TV_ITERS):
                if it == 0:
                    # p=0: duals from raw grads of xcb (bf16 cast in tv1),
                    # lam1-normalized
                    nc.scalar.copy(F6(tv1), F6(xcb))
                    yield 1800
                    for ch in (0, 1):
                        def c_it0(ti, pv, mp0, mp1, ch=ch):
                            nc.scalar.square(
                                sv[mp0:mp1, 3 * ch + ti, 0:320], pv)
                            if ti == 2:
                                nc.vector.tensor_copy(
                                    qy[mp0:mp1, 3 * ch + ti, 0:320], pv)
                            else:
                                nc.scalar.copy(
                                    qy[mp0:mp1, 3 * ch + ti, 0:320], pv)
                        stage_tv("dgt", tv1, True, ch, c_it0)
                        yield 1600
                    nc.vector.tensor_tensor(F6(qx, 2, 321), F6(tv1, 1, 320),
                                            F6(tv1, 0, 319), ALU.subtract)
                    yield
                    nc.vector.tensor_tensor(F6(tv1), F6(qx, 2, 322),
                                            F6(qx, 2, 322), ALU.mult)
                    yield
                    nc.vector.tensor_tensor(F6(n2), F6(tv1), F6(sv), ALU.add)
                    yield
                    nc.vector.tensor_scalar_max(F6(n2), F6(n2), lam1 * lam1)
                    nc.scalar.activation(F6(n2), F6(n2),
                                         AF.Abs_reciprocal_sqrt,
                                         scale=1.0 / (lam * lam))
                    yield 2300
                    nc.vector.tensor_tensor(F6(qx, 2, 321), F6(qx, 2, 321),
                                            F6(n2, 0, 319), ALU.mult)
                    yield
                    ew(TT(ALU.mult), pc(qy), pc(qy), pc(n2), pat="vg")
                    yield
                else:
                    # sv = s*(xcb - div_w(qx) - div_h(qy)) via one fold group
                    for ch in (0, 1):
                        stage_tv("ddt", qy, False, ch,
                                 scale_pv(sv, ch, -s_tv, pat="aav"),
                                 fold=W_FOLD)
                        yield 1800
                    # qy = I*qy + Dg*sv (fold);  qx += grad_w(sv)
                    stage_tv("dgt", sv, True, 0, cp_pv(qy, 0, pat="ava"),
                             fold=[("idp", qy, 0)])
                    yield 1500
                    nc.vector.tensor_tensor(F6(tv1, 0, 319), F6(sv, 1, 320),
                                            F6(sv, 0, 319), ALU.subtract)
                    yield
                    stage_tv("dgt", sv, True, 1, cp_pv(qy, 1, pat="ava"),
                             fold=[("idp", qy, 0)])
                    yield 1500
                    nc.vector.tensor_tensor(F6(qx, 2, 321), F6(qx, 2, 321),
                                            F6(tv1, 0, 319), ALU.add)
                    yield
                    nc.vector.tensor_tensor(F6(tv1), F6(qx, 2, 322),
                                            F6(qx, 2, 322), ALU.mult)
                    nc.scalar.square(F6(n2), F6(qy))
                    yield
                    nc.vector.tensor_tensor(F6(n2), F6(n2), F6(tv1), ALU.add)
                    yield
                    nc.vector.tensor_scalar_max(F6(n2), F6(n2), lam * lam)
                    nc.scalar.activation(F6(n2), F6(n2),
                                         AF.Abs_reciprocal_sqrt,
                                         scale=1.0 / (lam * lam))
                    yield 2300
                    nc.vector.tensor_tensor(F6(qx, 2, 321), F6(qx, 2, 321),
                                            F6(n2, 0, 319), ALU.mult)
                    yield
                    ew(TT(ALU.mult), pc(qy), pc(qy), pc(n2), pat="vg")
                    yield
            # x_tv = xcb - div(q): same fold group, scale -1
            for ch in (0, 1):
                stage_tv("ddt", qy, False, ch,
                         scale_pv(xtv, ch, -1.0, pat="ava"), fold=W_FOLD)
                yield 1800

        def C_phase(i, k):
            """3-level Haar DWT soft-threshold of xtv -> xnew."""
            im = per_img[i]
            xnew = im["xB"] if k % 2 == 0 else im["xA"]
            if SKIP_DWT:
                ew(CP, pc(xnew), pc(xtv), pat="vg")
                yield
                return
            ew(TT(ALU.add), pc(wdwt, 0, 160), pc(xtv, 0, 320, 2),
               pc(xtv, 1, 320, 2), pat="vg")
            ew(TT(ALU.subtract), pc(wdwt, 160, 320), pc(xtv, 0, 320, 2),
               pc(xtv, 1, 320, 2), pat="gg")
            yield
            for ch in (0, 1):
                stage_w1("w1t", wdwt, ch, cp_pv(Y1, ch, pat="vav"))
                yield 2400
            segeng = os.environ.get("CS_SEG", "vgg")
            for ch in (0, 1):
                for si, (sp, sq, dp, dq, cnt) in enumerate(SEG12[ch]):
                    e = ENG[segeng[(ch + si) % 3]]
                    e.tensor_tensor(L2t[dp:dp + cnt, dq, 0:80],
                                    Y1[sp:sp + cnt, sq, 0:160:2],
                                    Y1[sp:sp + cnt, sq, 1:160:2], ALU.add)
                    e.tensor_tensor(L2t[dp:dp + cnt, dq, 80:160],
                                    Y1[sp:sp + cnt, sq, 0:160:2],
                                    Y1[sp:sp + cnt, sq, 1:160:2], ALU.subtract)
            yield
            for ch in (0, 1):
                pY2 = psB()
                mm_lvl(pY2, "w2t", L2t, L2D, L2C, 160, ch)
                nc.scalar.copy(Y2[0:128, 2 * ch, :], pY2[0:128, 0, 0:160])
                nc.scalar.copy(Y2[0:32, 2 * ch + 1, :], pY2[0:32, 1, 0:160])
            for ch in (0, 1):
                for si, (sp, sq, dp, dq, cnt) in enumerate(SEG23[ch]):
                    e = ENG[segeng[(ch + si) % 3]]
                    e.tensor_tensor(L3t[dp:dp + cnt, dq, 0:40],
                                    Y2[sp:sp + cnt, sq, 0:80:2],
                                    Y2[sp:sp + cnt, sq, 1:80:2], ALU.add)
                    e.tensor_tensor(L3t[dp:dp + cnt, dq, 40:80],
                                    Y2[sp:sp + cnt, sq, 0:80:2],
                                    Y2[sp:sp + cnt, sq, 1:80:2], ALU.subtract)
            yield
            for ch in (0, 1):
                pY3 = psB()
                mm_lvl(pY3, "w3t", L3t, L3D, L3C, 80, ch)
                nc.scalar.copy(Y3[0:64, 2 * ch, :], pY3[0:64, 0, 0:80])
                nc.scalar.copy(Y3[0:16, 2 * ch + 1, :], pY3[0:16, 1, 0:80])
            # thresholds: save ll3, soft-threshold everything, restore ll3
            nc.scalar.copy(L3t[0:40, 0:4:2, 0:40], Y3[0:40, 0:4:2, 0:40])
            soft_views([Y3[0:64, 0:4:2, :], Y3[0:16, 1:4:2, :]], lam_lvl[2],
                       [wdwt[0:64, 0:4:2, 0:80], wdwt[0:16, 1:4:2, 0:80]],
                       pat="vg")
            nc.scalar.copy(Y3[0:40, 0:4:2, 0:40], L3t[0:40, 0:4:2, 0:40])
            soft_views([Y2[0:128, 0:4:2, :], Y2[0:32, 1:4:2, :]], lam_lvl[1],
                       [wdwt[0:128, 0:4:2, 0:160], wdwt[0:32, 1:4:2, 0:160]],
                       pat="gv")
            yield
            soft_views(pc(Y1), lam_lvl[0], pc(wdwt), pat="gg")
            yield
            # ---------- inverse ----------
            for ch in (0, 1):
                pZ3 = psB()
                mm_lvl(pZ3, "w3h", Y3, L3D, L3C, 80, ch)
                nc.scalar.copy(L3t[0:64, 2 * ch, :], pZ3[0:64, 0, 0:80])
                nc.scalar.copy(L3t[0:16, 2 * ch + 1, :], pZ3[0:16, 1, 0:80])
            for ch in (0, 1):
                for si, (sp, sq, dp, dq, cnt) in enumerate(SEG23[ch]):
                    e = ENG[segeng[(ch + si) % 3]]
                    e.tensor_tensor(Y2[sp:sp + cnt, sq, 0:80:2],
                                    L3t[dp:dp + cnt, dq, 0:40],
                                    L3t[dp:dp + cnt, dq, 40:80], ALU.add)
                    e.tensor_tensor(Y2[sp:sp + cnt, sq, 1:80:2],
                                    L3t[dp:dp + cnt, dq, 0:40],
                                    L3t[dp:dp + cnt, dq, 40:80], ALU.subtract)
            yield
            for ch in (0, 1):
                pZ2 = psB()
                mm_lvl(pZ2, "w2h", Y2, L2D, L2C, 160, ch)
                nc.scalar.copy(L2t[0:128, 2 * ch, :], pZ2[0:128, 0, 0:160])
                nc.scalar.copy(L2t[0:32, 2 * ch + 1, :], pZ2[0:32, 1, 0:160])
            for ch in (0, 1):
                for si, (sp, sq, dp, dq, cnt) in enumerate(SEG12[ch]):
                    e = ENG[segeng[(ch + si) % 3]]
                    e.tensor_tensor(Y1[sp:sp + cnt, sq, 0:160:2],
                                    L2t[dp:dp + cnt, dq, 0:80],
                                    L2t[dp:dp + cnt, dq, 80:160], ALU.add)
                    e.tensor_tensor(Y1[sp:sp + cnt, sq, 1:160:2],
                                    L2t[dp:dp + cnt, dq, 0:80],
                                    L2t[dp:dp + cnt, dq, 80:160], ALU.subtract)
            yield
            for ch in (0, 1):
                stage_w1("w1h", Y1, ch, cp_pv(wdwt, ch, pat="avv"))
                yield 2400
            ew(TT(ALU.add), pc(xnew, 0, 320, 2), pc(wdwt, 0, 160),
               pc(wdwt, 160, 320), pat="gg")
            ew(TT(ALU.subtract), pc(xnew, 1, 320, 2), pc(wdwt, 0, 160),
               pc(wdwt, 160, 320), pat="vg")
            yield

        def D_phase(i, k):
            """FISTA momentum: z = (1+c_k) xnew - c_k xold (no scratch)."""
            im = per_img[i]
            xold = im["xA"] if k % 2 == 0 else im["xB"]
            xnew = im["xB"] if k % 2 == 0 else im["xA"]
            if k < MAX_ITER - 1:
                ew(TT(ALU.subtract), pc(im["tv1"]), pc(xnew), pc(xold),
                   pat="gg")
                yield 2100
                nc.vector.scalar_tensor_tensor(F6(im["z"]), F6(im["tv1"]),
                                               coeffs[k], F6(xnew),
                                               ALU.mult, ALU.add)
            yield 2100

        def CD(i, k):
            yield from C_phase(i, k)
            yield from D_phase(i, k)

        def chain(*gens):
            for g in gens:
                if g is not None:
                    yield from g

        def wfq(*streams):
            """Weighted-fair merge: always advance the stream with the least
            emitted virtual time (chunk yields estimate their cost in ns)."""
            act = [[0.0, i, g] for i, g in enumerate(streams) if g is not None]
            while act:
                ent = min(act)
                try:
                    c = next(ent[2])
                    ent[0] += float(c) if isinstance(c, (int, float)) else 1050.0
                except StopIteration:
                    act.remove(ent)

        # ---------- FISTA: 2-image software pipeline ----------
        # One continuous per-image stream each (B->C->D->A per iteration),
        # merged by virtual-time WFQ; image 1 leads with its A phase so the
        # streams sit roughly half a phase apart (PE-heavy vs DVE-heavy).
        def img_stream(i):
            for k in range(MAX_ITER):
                yield from B_phase(i)
                yield from C_phase(i, k)
                yield from D_phase(i, k)
                if k < MAX_ITER - 1:
                    yield from A_phase(i)

        for _ in A_phase(0):
            pass
        wfq(img_stream(0), chain(A_phase(1), img_stream(1)))

        fin = "xB" if (MAX_ITER - 1) % 2 == 0 else "xA"
        for i in range(IMGS):
            nc.sync.dma_start(dr[f"xo{i}"][:], per_img[i][fin][:])

    nc.compile()
    return nc


_NC = None


def _get_nc():
    global _NC
    if _NC is None:
        _NC = _build_nc()
    return _NC


def kernel(y, mask):
    from concourse.bass_utils import run_bass_kernel_spmd

    y = np.asarray(y, dtype=np.float32)
    mask = np.asarray(mask, dtype=np.float32)
    nc = _get_nc()
    in_maps = build_in_maps(y, mask)

    res = run_bass_kernel_spmd(nc, in_maps, list(range(NCORES)))
    global LAST_RES
    LAST_RES = res
    out = np.zeros((B, 2, H, W), dtype=np.float32)
    for core in range(NCORES):
        for i in range(IMGS):
            out[core * IMGS + i] = _unpack_p6(res.results[core][f"xo{i}"])
    return out

